# revision 15
# baseline (speedup 1.0000x reference)
"""AdaptiveConv Trainium2 kernel — SWDGE dma_gather version.

Strategy (data-parallel over batch, one batch element per NeuronCore):
  out[o,h,w] = sum_{t=9 taps} W_t[o,i] . bilinear_sample(x, sy[h]+m_t*dil,
  sx[w]+n_t*dil)[i,h,w].

Device pipeline per core, per half-strip of 4 output rows (1024 px):
  1. One gpsimd dma_gather (SWDGE): 9216 int16 indices (9 taps x 1024 px),
     each fetching a 512 B "quad" row from an HBM table
     Q[pos] = [ch(64), corner(4)] bf16 (the 2x2 bilinear patch for all 64
     channels at grid position pos). Pixels land px-on-partition:
     gout[p = px%128, slot = tap*8 + px//128, 256].
  2. DVE: one in-place tensor_tensor mult per tap with the 4 per-pixel
     bilinear corner weights (free-dim broadcast across channels; innermost
     dim contiguous bf16 so the DVE runs in packed 2x mode).
  3. PE: two K=128 matmuls per (tap, slot) against a 128x128 identity
     transpose the weighted quads to ch-on-partition and SUM the 4 corners
     in f32 PSUM: psT rows = (ch*2 + corner-pair parity).
  4. Scalar: copy psT (f32) -> sampT (bf16 SBUF).
  5. PE: channel-mix matmuls with row-duplicated weights
     wt[(ch,par), o] = W_t[o, ch] (K=128 sums the two parities for free),
     accumulating all 9 taps into [64, 512] PSUM chunks.
  6. Scalar copies PSUM -> SBUF f32; DMA to the output.

All coordinate math runs on the host: the fixed problem inputs are baked
into per-half-strip int16 index tables (idx), per-pixel corner weights
(w4), and the Q table. Per-half-strip Q-window base offsets are baked into
the NEFF (identical for all 8 cores; cached on those bases).
"""
import sys

sys.path.insert(0, "/opt/trn_rl_repo")

import numpy as np

from concourse import bacc, bass, mybir
from concourse import bass_utils
from concourse.tile import TileContext

F32 = mybir.dt.float32
BF16 = mybir.dt.bfloat16
I16 = mybir.dt.int16

B, C, H, W = 8, 64, 256, 256
PAD = 3
PH = H + 2 * PAD      # 262
PW = W + 2 * PAD      # 262
TAPS = 9
PXH = 1024            # pixels per half-strip (4 output rows)
HSN = (H * W) // PXH  # 64 half-strips
NIDX = TAPS * PXH     # 9216 gather indices per half-strip
ELEM = 4 * C          # 256 bf16 values (512 B) per gather index

_CACHE = {}


def _build(bases, wins, n_hs=HSN):
    nc = bacc.Bacc("TRN2", target_bir_lowering=True, num_swdge_queues=4)
    q_in = nc.declare_dram_parameter("q", [PH * PW, ELEM], BF16, isOutput=False)
    idx_in = nc.declare_dram_parameter("idx", [HSN, 128, NIDX // 16], I16,
                                       isOutput=False)
    w4_in = nc.declare_dram_parameter("w4", [HSN, 128, TAPS, 16, 2], BF16,
                                      isOutput=False)
    wt_in = nc.declare_dram_parameter("wt", [128, TAPS * 64], BF16,
                                      isOutput=False)
    id_in = nc.declare_dram_parameter("ident", [128, 128], BF16, isOutput=False)
    out = nc.declare_dram_parameter("out", [C, H * W], F32, isOutput=True)

    with TileContext(nc) as tc:
        with tc.tile_pool(name="pstat", bufs=1) as pstat, \
             tc.tile_pool(name="pg", bufs=3) as pg, \
             tc.tile_pool(name="pi", bufs=2) as pi, \
             tc.tile_pool(name="pw", bufs=2) as pw, \
             tc.tile_pool(name="pt", bufs=3) as pt, \
             tc.tile_pool(name="po", bufs=2) as po, \
             tc.tile_pool(name="ps", bufs=2, space="PSUM") as ps, \
             tc.tile_pool(name="pso", bufs=2, space="PSUM") as pso:
            wt_b = pstat.tile([128, TAPS * 64], BF16, tag="wtb")
            nc.sync.dma_start(out=wt_b[:], in_=wt_in[:])
            id_b = pstat.tile([128, 128], BF16, tag="idb")
            nc.sync.dma_start(out=id_b[:], in_=id_in[:])

            for hs in range(n_hs):
                idxt = pi.tile([128, NIDX // 16], I16, tag="idx", name=f"i{hs}")
                nc.sync.dma_start(out=idxt[:], in_=idx_in[hs])
                w4t = pw.tile([128, TAPS, 16, 2], BF16, tag="w4", name=f"w{hs}")
                nc.sync.dma_start(out=w4t[:], in_=w4_in[hs])

                gout = pg.tile([128, TAPS * 8, ELEM], BF16, tag="g",
                               name=f"g{hs}")
                src = q_in[bases[hs] * PW:(bases[hs] + wins[hs]) * PW, :]
                for t in range(TAPS):
                    nc.gpsimd.dma_gather(
                        gout[:, t * 8:(t + 1) * 8, :], src,
                        idxt[:, t * 64:(t + 1) * 64],
                        PXH, PXH, ELEM, queue_num=(hs * TAPS + t) % 4)

                po0 = pso.tile([64, 512], F32, tag="po0", name=f"po0_{hs}")
                po1 = pso.tile([64, 512], F32, tag="po1", name=f"po1_{hs}")
                # elem layout [j(2), ch(64), c(2)]: (slot, j) fused -> sj
                gv = gout[:].rearrange("p s (j c q) -> p (s j) c q", j=2, c=C)
                gm = gout[:].rearrange("p s (j m) -> p s j m", j=2)
                for t in range(TAPS):
                    gt = gv[:, t * 16:(t + 1) * 16, :, :]
                    w4b = (w4t[:, t]
                           .rearrange("p sj (u q) -> p sj u q", u=1)
                           .broadcast_to((128, 16, C, 2)))
                    nc.vector.tensor_tensor(gt, gt, w4b, mybir.AluOpType.mult)

                    psT = ps.tile([128, 1024], F32, tag="psT",
                                  name=f"t{hs}_{t}")
                    for k in range(8):
                        for jp in range(2):
                            nc.tensor.matmul(
                                psT[:, k * 128:(k + 1) * 128],
                                gm[:, t * 8 + k, jp, :],
                                id_b[:],
                                start=(jp == 0), stop=(jp == 1))
                    sampT = pt.tile([128, 1024], BF16, tag="sT",
                                    name=f"s{hs}_{t}")
                    nc.scalar.copy(out=sampT[:], in_=psT[:])
                    for cc, pot in enumerate((po0, po1)):
                        nc.tensor.matmul(
                            pot[:],
                            wt_b[:, t * 64:(t + 1) * 64],
                            sampT[:, cc * 512:(cc + 1) * 512],
                            start=(t == 0), stop=(t == TAPS - 1))

                obuf = po.tile([64, 1024], F32, tag="ob", name=f"o{hs}")
                nc.scalar.copy(out=obuf[:, 0:512], in_=po0[:])
                nc.scalar.copy(out=obuf[:, 512:1024], in_=po1[:])
                nc.sync.dma_start(out=out[:, hs * PXH:(hs + 1) * PXH],
                                  in_=obuf[:])
    nc.finalize()
    return nc


def _coords(sh_b, sw_b, dil_b):
    """Per-core tap coordinates: padded row/col of the bilinear floor and the
    4 corner weights. Returns (rowp, colp [9,H,W] i32, w4 [9,H,W,4] f32)."""
    f32 = np.float32
    sy = (sh_b.astype(f32) + f32(1.0)) * f32((H - 1) / 2.0)
    sx = (sw_b.astype(f32) + f32(1.0)) * f32((W - 1) / 2.0)
    d = dil_b.astype(f32).reshape(H, W)
    rowp = np.empty((TAPS, H, W), np.int32)
    colp = np.empty((TAPS, H, W), np.int32)
    w4 = np.empty((TAPS, H, W, 4), f32)
    for kh in range(3):
        yy = sy[:, None] + f32(kh - 1) * d
        y0 = np.floor(yy)
        fy = yy - y0
        for kw in range(3):
            xx = sx[None, :] + f32(kw - 1) * d
            x0 = np.floor(xx)
            fx = xx - x0
            t = kh * 3 + kw
            rowp[t] = y0.astype(np.int32) + PAD
            colp[t] = x0.astype(np.int32) + PAD
            w4[t, :, :, 0] = (1 - fy) * (1 - fx)
            w4[t, :, :, 1] = (1 - fy) * fx
            w4[t, :, :, 2] = fy * (1 - fx)
            w4[t, :, :, 3] = fy * fx
    return rowp, colp, w4


def _prep(x, sh, sw, dil, wgt):
    """Host-side metadata: returns (bases, wins, in_maps)."""
    import ml_dtypes
    bf16 = ml_dtypes.bfloat16

    coords = [_coords(sh[b], sw[b], dil[b]) for b in range(B)]

    # shared per-half-strip Q-window bases (min padded row over all cores)
    rows_all = np.stack([c[0] for c in coords])          # [B, 9, H, W]
    rmin = rows_all.reshape(B, TAPS, HSN, PXH).min(axis=(0, 1, 3))
    rmax = rows_all.reshape(B, TAPS, HSN, PXH).max(axis=(0, 1, 3))
    bases = tuple(int(v) for v in rmin)
    wins = tuple(int(v) for v in (rmax - rmin + 1))
    assert max(wins) * PW < 2 ** 15

    # channel-mix weights, rows duplicated per corner-pair parity
    wt9 = wgt.transpose(2, 3, 1, 0).reshape(TAPS, C, C)   # [t, i, o]
    wtd = np.repeat(wt9, 2, axis=1)                       # [t, 128, 64]
    wt_host = np.ascontiguousarray(
        wtd.transpose(1, 0, 2).reshape(128, TAPS * 64)).astype(bf16)
    ident = np.eye(128, dtype=np.float32).astype(bf16)

    in_maps = []
    for b in range(B):
        rowp, colp, w4 = coords[b]
        # Q table: Q[r, w, j, ch, c] = xpad[ch, r+j, w+c]
        xT = np.zeros((PH + 1, PW + 1, C), np.float32)
        xT[PAD:PAD + H, PAD:PAD + W, :] = x[b].transpose(1, 2, 0)
        xTb = xT.astype(bf16)
        Q = np.empty((PH, PW, 2, C, 2), bf16)
        Q[:, :, 0, :, 0] = xTb[:-1, :-1]
        Q[:, :, 0, :, 1] = xTb[:-1, 1:]
        Q[:, :, 1, :, 0] = xTb[1:, :-1]
        Q[:, :, 1, :, 1] = xTb[1:, 1:]

        # int16 index tables, wrapped-16 and replicated across the 8 Q7 cores
        rel = (rowp.reshape(TAPS, HSN, PXH)
               - np.asarray(bases, np.int32)[None, :, None]) * PW \
            + colp.reshape(TAPS, HSN, PXH)
        assert rel.min() >= 0 and rel.max() < 2 ** 15
        # per-tap wrapped-16 index blocks (one dma_gather per tap)
        idx_host = np.empty((HSN, 128, NIDX // 16), np.int16)
        for hs in range(HSN):
            for t in range(TAPS):
                blk = rel[t, hs, :].astype(np.int16)
                idx_host[hs][:, t * 64:(t + 1) * 64] = np.tile(
                    blk.reshape(64, 16).T, (8, 1))

        # corner weights in gather layout [hs, p, t, slot*j, c]
        w4g = np.ascontiguousarray(
            w4.reshape(TAPS, HSN, 8, 128, 4).transpose(1, 3, 0, 2, 4)
        ).astype(bf16).reshape(HSN, 128, TAPS, 16, 2)

        in_maps.append({
            "q": np.ascontiguousarray(Q.reshape(PH * PW, ELEM)),
            "idx": idx_host,
            "w4": w4g,
            "wt": wt_host,
            "ident": ident,
        })
    return bases, wins, in_maps


def kernel(x, stride_h, stride_w, dilation, weight):
    x = np.ascontiguousarray(np.asarray(x, dtype=np.float32))
    sh = np.asarray(stride_h, dtype=np.float32)
    sw = np.asarray(stride_w, dtype=np.float32)
    dil = np.asarray(dilation, dtype=np.float32)[:, 0]
    wgt = np.asarray(weight, dtype=np.float32)

    bases, wins, in_maps = _prep(x, sh, sw, dil, wgt)
    key = (bases, wins)
    if key not in _CACHE:
        _CACHE[key] = _build(bases, wins)
    nc = _CACHE[key]

    import os
    trace = bool(os.environ.get("AC_TRACE"))
    res = bass_utils.run_bass_kernel_spmd(nc, in_maps, core_ids=list(range(B)),
                                          trace=trace)
    if trace:
        kernel.last_exec_time_ns = res.exec_time_ns
    outp = np.stack([res.results[b]["out"].reshape(C, H, W) for b in range(B)])
    return outp
